# revision 49
# baseline (speedup 1.0000x reference)
"""Trainium2 Bass kernel for a grouped contrastive loss.

Math (matches the reference):
    z_a = concat(z_target, z_source)                      # [A=M+N, D]
    sims[a, j] = (z_a[a] . z_target[j]) / T
    den[j]  = sum_a exp(sims[a, j]) - exp(z_tj.z_tj / T)
    num[j]  = mean_{s: seg_source[s]==seg_target[j]} (z_s . z_tj) / T
            = (v_j . z_tj)   with v_j = S[seg_target[j]] / (count * T)
    loss = sum_j log(den[j]) - num[j]

Sharding: j-blocks strided across 8 cores (core c owns 128-col blocks
{8k+c}); z_a replicated. Key speedups over the ACT-only baseline:

1. Two-engine PSUM drain (only ACT and DVE may touch PSUM on TRN2): each
   2048-wide tile of sims is built in two single-consumer PSUM pools:
   - psA: ACT native Exp + accum_out              (0.833 ns/elem + 372)
   - psD: DVE Schraudolph exp -> int16            (1.042 ns/elem + 125)
   The Schraudolph affine y = s*K1 + K2, int16(y) bitcast to bf16, IS
   exp(s/T) to ~2% per element (mean error ~0 by K2 calibration); the
   bf16 spans are summed by DVE tensor_scalar+accum_out in 4x packed
   mode (0.26 ns/elem). Single-consumer pools matter: the tile scheduler
   chains all readers of a shared tile into a serial semaphore chain.
2. Symmetry: the zt x zt block of sims is symmetric, so slots 2/3
   (j-blocks >= 16) skip a-tile g=0 entirely; the missing terms are the
   partition-axis sums of slots 0/1's g=1 exp tiles, which the
   otherwise-idle Pool engine computes (tensor_reduce axis=C, its only
   PSUM-free reduce) into resC, redistributed across cores on the host.

The diagonal exp(z_tj.z_tj/T) ~ e^14.3 dwarfs den, so the host must
subtract (nearly) exactly what the device added. Schraudolph's int16
rounding would amplify a 2-ulp fp32 mismatch catastrophically, so the
diagonal MUST flow through the ACT path: each core's za is permuted so
its 4 diagonal blocks land at fixed positions inside psA chunks. Host
replicates the fp16-product fp32-pairwise dot (~2 ulp of the PE
accumulator) + np.exp, as in the proven baseline.

Host: tiny final reduction (log over 4096 columns + scalar sums) in float64.
"""

import numpy as np

TEMPERATURE = 0.07
N = 4096       # z_source rows
M = 4096       # z_target rows
D = 128        # embedding dim
G = 64         # groups
NCORES = 8
MLOC = M // NCORES          # 512 target columns per core
A = M + N                   # 8192 rows of z_a
W = 2048                    # a-columns per PSUM tile (4 banks)
NT = A // W                 # 4 a-tiles
NJB = MLOC // 128           # 4 column blocks of 128 per core
ACH = 512                   # matmul rhs width (one PSUM bank)
# HW constraints (BIR verifier): GPSIMD/Pool cannot access PSUM, and DMA
# cannot read PSUM -- so only ACT and DVE can drain the matmul results.
# Each logical 2048-wide tile is built in TWO separate PSUM pools with ONE
# consumer each (the tile scheduler chains all readers of a shared tile
# into a serial semaphore chain, which halves pipeline throughput):
#   psA [128,1024] (a-cols 0:1024, incl the diagonal): ACT native exp
#       + accum_out.
#   psD [128,1024] (a-cols 1024:2048): DVE Schraudolph affine -> int16.
# DVE 4x-reduces the bf16 spans at 0.26 ns/el; Pool's contribution is the
# SBUF-side partition-axis C-reduces for the symmetry scheme below.
CH = 1024                    # chunk width (2 PSUM banks)
# Symmetry: the z_t x z_t block of sims is symmetric, so pairs only need
# computing once. Core c owns j-blocks {c, 8+c, 16+c, 24+c} (slot k ->
# block 8k+c). Slots 2/3 (j-blocks >= 16) SKIP a-tile g=0 (zt blocks
# 0-15): those contributions are recovered from the partition-axis
# (C-axis) sums of slots 0/1's g=1 tiles, computed by the otherwise-idle
# Pool engine (the only reduce it can legally do) and redistributed on
# the host. 14 tiles remain of 16.
TILES = [(g, k) for g in range(4) for k in range(4)
         if not (g == 0 and k >= 2)]
TIDX = {t: i for i, t in enumerate(TILES)}
# Tiles where ACT consumes BOTH chunks (psA and psD) for load balance;
# their q1/q2 spans are empty.
ACT_BOTH = {(2, 2): 0, (3, 3): 1}
# Schraudolph constants: y = s*K1 + K2; int16(y) bitcast bf16 ~= exp(s/T).
# K2 calibrated so the mean per-element error over uniform exponent
# fractions is ~0; the HW fp32->int16 conversion rounds to nearest
# (verified on device), for which the zero-bias constant is 16248.75.
K1 = float(np.float32(128.0 * np.log2(np.e) / TEMPERATURE))
K2 = 16248.75

_CACHE = {}


def _build_bass():
    import concourse.mybir as mybir
    from concourse import bacc
    from concourse.tile import TileContext

    f32 = mybir.dt.float32
    f16 = mybir.dt.float16
    bf16 = mybir.dt.bfloat16
    i16 = mybir.dt.int16
    Alu = mybir.AluOpType

    nc = bacc.Bacc("TRN2", num_devices=NCORES)
    zaT = nc.dram_tensor("zaT", [D, A], f16, kind="ExternalInput")
    ztT = nc.dram_tensor("ztT", [D, MLOC], f16, kind="ExternalInput")
    vtT = nc.dram_tensor("vtT", [D, MLOC], f32, kind="ExternalInput")
    resA = nc.dram_tensor("resA", [128, len(TILES) + 2], f32,
                          kind="ExternalOutput")
    resC = nc.dram_tensor("resC", [1, 4 * CH], f32, kind="ExternalOutput")
    # 3 reduce cols per jb (pair g0+g1, g2, g3 — the per-g split keeps the
    # final reduce short, off the drain critical path) + 1 num col.
    resR = nc.dram_tensor("resR", [128, 3 * NJB + 1], f32, kind="ExternalOutput")

    with TileContext(nc) as tc:
        with (
            tc.tile_pool(name="persist", bufs=1) as persist,
            tc.tile_pool(name="psumA", bufs=2, space="PSUM") as poolA,
            tc.tile_pool(name="psumD", bufs=2, space="PSUM") as poolD,
        ):
            # DMA order = consumption order, smallest-first so the first
            # matmul's data (zt block 0 + za cols 0:512) clears the
            # HWDGE+DGE+sem latency ladder as early as possible.
            zt_tile = persist.tile([128, MLOC], f16, tag="zt")
            za_tiles = [persist.tile([128, W], f16, tag=f"za{g}",
                                     name=f"za{g}") for g in range(NT)]
            nc.sync.dma_start(out=zt_tile[:, 0:128], in_=ztT[:, 0:128])
            nc.sync.dma_start(out=za_tiles[0][:, 0:512], in_=zaT[:, 0:512])
            nc.sync.dma_start(out=za_tiles[0][:, 512:1024],
                              in_=zaT[:, 512:1024])
            nc.sync.dma_start(out=za_tiles[0][:, 1024:2048],
                              in_=zaT[:, 1024:2048])
            nc.sync.dma_start(out=zt_tile[:, 128:MLOC], in_=ztT[:, 128:MLOC])
            vt_tile = persist.tile([128, MLOC], f32, tag="vt")
            nc.sync.dma_start(out=vt_tile[:], in_=vtT[:, :])
            for g in range(1, NT):
                nc.sync.dma_start(out=za_tiles[g][:],
                                  in_=zaT[:, g * W:(g + 1) * W])

            scratch = [persist.tile([128, CH], f32, tag=f"scr{i}",
                                    name=f"scr{i}") for i in range(2)]
            resA_t = persist.tile([128, len(TILES) + 2], f32, tag="resA")
            resC_t = persist.tile([1, 4 * CH], f32, tag="resC")
            cexp = [persist.tile([128, CH], bf16, tag=f"cexp{k}",
                                 name=f"cexp{k}") for k in range(2)]
            resR_t = persist.tile([128, 3 * NJB + 1], f32, tag="resR")
            # The two ACT_BOTH spans never write their resR cols; zero the
            # tile so the output DMA reads defined data.
            nc.gpsimd.memset(resR_t[:], 0.0)
            # One SBUF span-tile per reduce, so a reduce's only dependency
            # is its own writers (shared tiles get reader/writer-chained by
            # the scheduler, convoying the pipeline).
            spans = {}
            for jb in range(NJB):
                w0 = 2 * CH if jb < 2 else CH
                spans[(0, jb)] = persist.tile(
                    [128, w0], i16, tag=f"sp0_{jb}", name=f"sp0_{jb}")
                spans[(1, jb)] = persist.tile(
                    [128, CH], i16, tag=f"sp1_{jb}", name=f"sp1_{jb}")
                spans[(2, jb)] = persist.tile(
                    [128, CH], i16, tag=f"sp2_{jb}", name=f"sp2_{jb}")
            nscr = persist.tile([128, MLOC], f32, tag="nscr")

            def emit_tile(g, jb):
                ti = TIDX[(g, jb)]
                psa = poolA.tile([128, CH], f32, tag="psa")
                psd = poolD.tile([128, CH], f32, tag="psd")
                lhsT = zt_tile[:, jb * 128:(jb + 1) * 128]
                za = za_tiles[g]
                # psd first: DVE's affine releases sooner than ACT's chain,
                # so when psa's WAR gate stalls PE's in-order queue the psd
                # chunk is already issued.
                nc.tensor.matmul(psd[:, 0:512], lhsT, za[:, 1024:1536],
                                 start=True, stop=True)
                nc.tensor.matmul(psd[:, 512:1024], lhsT, za[:, 1536:2048],
                                 start=True, stop=True)
                nc.tensor.matmul(psa[:, 0:512], lhsT, za[:, 0:512],
                                 start=True, stop=True)
                nc.tensor.matmul(psa[:, 512:1024], lhsT, za[:, 512:1024],
                                 start=True, stop=True)
                # C-tiles (g=1, slots 0/1) keep their exps (bf16) for the
                # Pool partition-sum; others discard into scratch.
                ctile = g == 1 and jb < 2
                aout = cexp[jb][:] if ctile else scratch[ti % 2][:]
                nc.scalar.activation(
                    out=aout, in_=psa[:],
                    func=mybir.ActivationFunctionType.Exp,
                    scale=1.0 / TEMPERATURE,
                    accum_out=resA_t[:, ti:ti + 1])
                if (g, jb) in ACT_BOTH:
                    # ACT also eats this tile's psD chunk (load balance:
                    # DVE carries all reduces), into an extra accum col.
                    xcol = len(TILES) + ACT_BOTH[(g, jb)]
                    nc.scalar.activation(
                        out=scratch[ti % 2][:], in_=psd[:],
                        func=mybir.ActivationFunctionType.Exp,
                        scale=1.0 / TEMPERATURE,
                        accum_out=resA_t[:, xcol:xcol + 1])
                    return
                if g < 2:
                    seg = spans[(0, jb)][:, (g if jb < 2 else 0) * CH:
                                         (g + 1 if jb < 2 else 1) * CH]
                else:
                    seg = spans[(g - 1, jb)][:]
                nc.vector.tensor_scalar(
                    out=seg, in0=psd[:],
                    scalar1=K1, scalar2=K2, op0=Alu.mult, op1=Alu.add)

            def emit_reduce(q, jb):
                # bf16 sum of a Schraudolph span in DVE 4x mode (0.26
                # ns/el): q=0 covers tiles (g=0,1), q=1 covers g=2, q=2
                # covers g=3. (Pool cannot use accumulators on HW.)
                bview = spans[(q, jb)][:].bitcast(bf16)
                nc.vector.tensor_scalar(
                    out=bview, in0=bview, scalar1=1.0, scalar2=None,
                    op0=Alu.mult, op1=Alu.add,
                    accum_out=resR_t[:, jb * 3 + q:jb * 3 + q + 1])

            # Reduce (q, jb) becomes ready after its last tile's affine:
            # q=0 after lin 4+jb, q=1 after 8+jb, q=2 after 12+jb. Emit a
            # slot later (Pool's queue holds only reduces; DVE's q2 ops are
            # the pipeline drain).
            def emit_cred(k):
                # Pool partition-axis sums of the (g=1, slot k) tile's exps:
                # ACT's bf16 psA exps and the bf16 Schraudolph psD segment.
                base = 2 * k * CH
                nc.gpsimd.tensor_reduce(
                    out=resC_t[0:1, base:base + CH], in_=cexp[k][:],
                    axis=mybir.AxisListType.C, op=Alu.add)
                seg = spans[(0, k)][:, CH:2 * CH].bitcast(bf16)
                nc.gpsimd.tensor_reduce(
                    out=resC_t[0:1, base + CH:base + 2 * CH], in_=seg,
                    axis=mybir.AxisListType.C, op=Alu.add)

            pending = {}
            for jb in range(NJB):
                pending.setdefault(TIDX[(1, jb)] + 2, []).append(("r", 0, jb))
                if (2, jb) not in ACT_BOTH:
                    pending.setdefault(TIDX[(2, jb)] + 2, []).append(("r", 1, jb))
                if (3, jb) not in ACT_BOTH:
                    pending.setdefault(TIDX[(3, jb)] + 1, []).append(("r", 2, jb))
            for k in range(2):
                pending.setdefault(TIDX[(1, k)] + 1, []).append(("c", k))
            for g in range(NT):
                for jb in range(NJB):
                    if (g, jb) not in TIDX:
                        continue
                    emit_tile(g, jb)
                    ti = TIDX[(g, jb)]
                    for item in pending.pop(ti, []):
                        if item[0] == "r":
                            emit_reduce(item[1], item[2])
                        else:
                            emit_cred(item[1])
                if g == 0:
                    # num partial: accum_out = sum_j vt[d, j] * zt[d, j]
                    # (partition-dim d summed on host). On DVE: Pool cannot
                    # use accumulators (BIR verifier).
                    nc.vector.scalar_tensor_tensor(
                        out=nscr[:], in0=vt_tile[:], scalar=1.0,
                        in1=zt_tile[:], op0=Alu.mult, op1=Alu.mult,
                        accum_out=resR_t[:, 3 * NJB:3 * NJB + 1])
                if g == 2:
                    nc.sync.dma_start(out=resA[:, 0:6], in_=resA_t[:, 0:6])
            for ti in sorted(pending):
                for item in pending[ti]:
                    if item[0] == "r":
                        emit_reduce(item[1], item[2])
                    else:
                        emit_cred(item[1])
            nc.sync.dma_start(out=resA[:, 6:], in_=resA_t[:, 6:])
            nc.sync.dma_start(out=resR[:, :], in_=resR_t[:, :])
            nc.sync.dma_start(out=resC[:, :], in_=resC_t[:, :])
    nc.compile()
    return nc


def _get_nc():
    if "nc" not in _CACHE:
        _CACHE["nc"] = _build_bass()
    return _CACHE["nc"]


def _perm(c):
    """Per-core za row permutation: core c's diagonal blocks (j-blocks
    8k+c at rows 1024k+128c) move to fixed positions: slot 0 -> 0,
    slot 1 -> 128 (tile g=0), slot 2 -> 2048, slot 3 -> 2176 (tile g=1).
    Swaps stay inside their tile, so tile 0 holds zt blocks 0-15
    (reordered) and tile 1 holds blocks 16-31 (reordered)."""
    p = np.arange(A)
    for t0, s0 in ((0, 0), (128, 1024), (2048, 2048), (2176, 3072)):
        src = s0 + 128 * c
        if src != t0:
            tmp = p[t0:t0 + 128].copy()
            p[t0:t0 + 128] = p[src:src + 128]
            p[src:src + 128] = tmp
    return p


def kernel(z_source, z_target, seg_source, seg_target):
    from concourse.bass_utils import run_bass_kernel_spmd

    zs = np.ascontiguousarray(z_source, dtype=np.float32)
    zt = np.ascontiguousarray(z_target, dtype=np.float32)
    seg_s = np.asarray(seg_source).astype(np.int64)
    seg_t = np.asarray(seg_target).astype(np.int64)

    # Host-side sharding prep (O(N*D), trivial next to the O(A*M*D) device
    # work). fp16 quantization of the unit-norm z's (~2.4e-4 rel) keeps the
    # exp-sum well within the tolerance while halving DMA volume.
    za = np.concatenate([zt, zs], axis=0)                 # [A, D]
    zaT16 = np.ascontiguousarray(za.T.astype(np.float16))  # [D, A]
    counts = np.bincount(seg_s, minlength=G).astype(np.float32)
    S = np.zeros((G, D), np.float32)
    np.add.at(S, seg_s, zs)
    v = S[seg_t] / (counts[seg_t] * np.float32(TEMPERATURE))[:, None]
    vT = np.ascontiguousarray(v.T)                        # [D, M]

    in_maps = []
    for c in range(NCORES):
        # core c owns j-blocks {8k+c}: gather their zt/vt columns
        jidx = np.concatenate([128 * (8 * k + c) + np.arange(128)
                               for k in range(NJB)])
        in_maps.append({
            "zaT": np.ascontiguousarray(zaT16[:, _perm(c)]),
            "ztT": np.ascontiguousarray(zaT16[:, jidx]),
            "vtT": np.ascontiguousarray(vT[:, jidx]),
        })

    nc = _get_nc()
    out = run_bass_kernel_spmd(nc, in_maps, core_ids=list(range(NCORES)))
    results = out.results

    # Host finish (float64). The self dot replicates the device matmul
    # bit-closely: fp16 inputs make each product exact in fp32, and np.sum's
    # fp32 pairwise accumulation lands within ~2 ulp of the PE accumulator.
    h = zaT16[:, :M].astype(np.float32)
    self_dot = np.sum(h * h, axis=0, dtype=np.float32).astype(np.float64)
    self_exp = np.exp(self_dot / TEMPERATURE)             # [M]
    den = np.zeros(M)
    num_tot = 0.0
    for c in range(NCORES):
        rA = results[c]["resA"].astype(np.float64)        # [128, 16]
        rR = results[c]["resR"].astype(np.float64)        # [128, 13]
        for k in range(NJB):
            tot = rR[:, 3 * k].copy()
            for q in (1, 2):
                if (q + 1, k) not in ACT_BOTH:
                    tot += rR[:, 3 * k + q]
            for g in range(NT):
                if (g, k) not in TIDX:
                    continue
                tot += rA[:, TIDX[(g, k)]]
                if (g, k) in ACT_BOTH:
                    tot += rA[:, len(TILES) + ACT_BOTH[(g, k)]]
            j = 128 * (8 * k + c) + np.arange(128)
            den[j] += tot - self_exp[j]
        num_tot += rR[:, 3 * NJB].sum()
        # symmetric compensation: partition-sums of this core's (g=1,
        # slots 0/1) tiles are the missing a<2048-tile contributions for
        # the high j's at za positions [2048:4096).
        rC = results[c]["resC"][0].astype(np.float64)     # [4096]
        pos = _perm(c)[2048:4096]                         # global zt rows
        den[pos[0:1024]] += rC[0:1024] + rC[2048:3072]    # psA parts
        den[pos[1024:2048]] += rC[1024:2048] + rC[3072:4096]  # psD parts
    loss = np.sum(np.log(den)) - num_tot
    return np.asarray(loss, dtype=np.float32)
